# revision 1
# baseline (speedup 1.0000x reference)
"""Trainium2 Bass kernel for nn_AstraloraLayer: y = (x @ W^T) * scale + x.

x: [16384, 1024] f32, w: [1048576] f32 (W = w.reshape(1024, 1024)),
scale: [1] f32.  Data-parallel over 8 NeuronCores: each core takes 2048
tokens; w and scale are replicated; no collectives needed.

Device layout: everything is computed transposed (y^T = W' @ x^T) so the
contraction dim d lands on SBUF partitions for both matmul operands with
zero on-device transposes.  The host passes x^T shards and W'^T where
W' = scale*W + I — folding the scalar scale AND the residual into the
weights makes the whole layer one matmul; the epilogue is a plain PSUM
drain (DVE copy + store).  Matmul operands are host-cast to bf16 (rel
err ~2e-3 vs the f32 reference; fp32 accumulation in PSUM), which also
halves input DMA traffic.

Block 0 runs k-outer across 8 PSUM banks so PE consumption matches DMA
arrival order (the first matmul waits on one 256 KB w chunk + one x
chunk, not the 6 MB working set); steady-state blocks run o-outer/
k-inner so each output chunk's PSUM drain pipelines behind the PE
instead of bunching at block end.  Six throwaway matmuls on zeroed
tiles pre-warm the PE's HAM clock gate during the DMA lead-in (input
sems only fire ~11.7us in, after the DMA write-receipt round trip).
w loads + y stores issue on the sync HWDGE queue, x loads on the
scalar HWDGE queue (DMA issue costs ~0.6us per 128-descriptor
instruction — two queues double the feed rate, and w/x ride batched
multi-chunk 3D-AP DMAs to keep bytes-per-issue high).
"""

import numpy as np

_N_TOKENS = 16384
_D = 1024
_N_CORES = 8
_TOK_PER_CORE = _N_TOKENS // _N_CORES  # 2048
_TOK_BLOCK = 512
_P = 128

# Compute dtype for the matmul operands: "bf16" halves input DMA traffic
# (host casts the shards) and double-pumps the PE moving operand;
# "f32r" is full fp32 storage with single-pass reduced-precision matmul.
_COMPUTE = "bf16"

_cache = {}


def _apply_tile_drain_patch():
    """This walrus build rejects any instruction carrying more than one
    sync wait ("Too many sync wait commands", CoreV3 setupSyncWait), but
    Tile's wait-assignment pass freely emits multi-wait instructions.
    Two patches:

    1. Wrap TileClockWait so that after assign_waits() every instruction
       with >1 wait keeps only its last wait, with the others moved onto
       freshly inserted same-engine NoOps placed just before it.
    2. Re-emit the TileContext exit drain the same way (it waits on every
       live semaphore at once and is created after assign_waits ran).
    """
    if _cache.get("patched"):
        return
    import bass_rust
    import concourse.mybir as mybir
    from concourse import tile
    from concourse.vector_clock import ScopedClock

    _Orig = tile.TileClockWait
    _counter = [0]

    def _split_multi_waits(ordered):
        for insts in ordered.values():
            out = []
            for inst in insts:
                si = inst.sync_info
                if si is not None and len(si.on_wait) > 1:
                    waits = list(si.on_wait)
                    for w in waits[:-1]:
                        _counter[0] += 1
                        nop = mybir.InstNoOp(
                            name=f"I-wsplit-{_counter[0]}", ins=[], outs=[]
                        )
                        nop.engine = inst.engine
                        nop.bass_nofuse = True
                        nop.sync_info = bass_rust.SyncInfo(
                            on_wait=[w], on_update=[]
                        )
                        out.append(nop)
                    si.on_wait = waits[-1:]
                out.append(inst)
            insts[:] = out

    class _SplitWaitClock:
        def __init__(self, tc, ordered, **kw):
            object.__setattr__(self, "_inner", _Orig(tc, ordered, **kw))
            object.__setattr__(self, "_ordered", ordered)

        def assign_waits(self, bb):
            r = self._inner.assign_waits(bb)
            _split_multi_waits(self._ordered)
            return r

        def __getattr__(self, n):
            return getattr(object.__getattribute__(self, "_inner"), n)

    tile.TileClockWait = _SplitWaitClock

    def _drain_and_barrier(self, tick_clock, wait_clock):
        drain_inst = self.nc.sync.drain()
        wait_clock.add_sem_waits(
            drain_inst.ins, ScopedClock({None: tick_clock.global_clock})
        )
        si = drain_inst.ins.sync_info
        if si is not None and len(si.on_wait) > 1:
            waits = list(si.on_wait)
            si.on_wait = waits[:1]
            for w in waits[1:]:
                nop = self.nc.sync.nop(nofuse=True, hint="drain_wait_spill")
                nop.ins.sync_info = bass_rust.SyncInfo(on_wait=[w], on_update=[])

        self.nc.all_engine_barrier()
        assert self.sems is not None
        popped = self.nc._tile_sem_poison_stack.pop()
        assert popped is self._sem_poison
        self.nc.clear_and_free_semaphores(list(self.sems.allocated().values()))
        self.nc.all_engine_barrier()

    tile.TileContext._drain_and_barrier = _drain_and_barrier
    _cache["patched"] = True


def _build_nc(compute=None):
    import concourse.bass as bass
    import concourse.mybir as mybir
    from concourse import tile

    compute = compute or _COMPUTE
    f32 = mybir.dt.float32
    cd = mybir.dt.bfloat16 if compute == "bf16" else mybir.dt.float32r
    KC = _D // _P  # 8 contraction chunks
    OC = _D // _P  # 8 output-row chunks
    NB = _TOK_PER_CORE // _TOK_BLOCK  # token blocks

    nc = bass.Bass()
    xT = nc.declare_dram_parameter("xT", [_D, _TOK_PER_CORE], cd, isOutput=False)
    wT = nc.declare_dram_parameter("wT", [_D, _D], cd, isOutput=False)
    yT = nc.declare_dram_parameter("yT", [_D, _TOK_PER_CORE], f32, isOutput=True)

    with tile.TileContext(nc) as tc:
        with (
            tc.tile_pool(name="wp", bufs=1) as wp,
            tc.tile_pool(name="xp", bufs=2) as xp,
            tc.tile_pool(name="yp", bufs=8) as yp,
            tc.tile_pool(name="ps", bufs=1, space="PSUM") as ps,
        ):
            # PE pre-warm: the HAM clock gate holds the PE at 1.2 GHz until
            # it has been busy ~3.4us.  Input data only becomes sem-visible
            # ~11.7us in (DMA completion sems fire after the ~2us write
            # receipt, not at last byte), so six throwaway matmuls on zeroed
            # tiles keep the PE busy from ~8.5us and it is at 2.4 GHz when
            # the real stream starts.  Measured: this beats both more warm
            # matmuls (overshoot) and none (11 cold real matmuls).
            warm_dt = mybir.dt.bfloat16  # f32r memset is invalid ISA
            warm_w = wp.tile([_P, _P], warm_dt, tag="warm_w")
            warm_x = wp.tile([_P, _TOK_BLOCK], warm_dt, tag="warm_x")
            nc.vector.memset(warm_w[:], 0.0)
            nc.vector.memset(warm_x[:], 0.0)
            warm_ps = ps.tile([_P, _TOK_BLOCK], f32, tag="ps7", name="warm_ps")
            for i in range(6):
                nc.tensor.matmul(
                    warm_ps[:], lhsT=warm_w[:], rhs=warm_x[:],
                    start=True, stop=True,
                )

            # Weights: 4 DMAs of two k-chunks each on the sync HWDGE queue.
            # DMA issue is ~0.6us per 128-descriptor instruction, so bigger
            # rows (not more instructions) is how the feed keeps up.
            wtiles = {}
            for k in (0, 1):
                # k=0/1 load as singles so the very first matmul only waits
                # on a 256 KB chunk
                wt = wp.tile([_P, _D], cd, tag=f"ws{k}", name=f"ws{k}")
                nc.sync.dma_start(out=wt[:], in_=wT[k * _P : (k + 1) * _P, :])
                wtiles[k] = (wt, 0)
            for j in range(1, KC // 2):
                wt = wp.tile([_P, 2 * _D], cd, tag=f"w{j}", name=f"w{j}")
                nc.sync.dma_start(
                    out=wt[:].rearrange("p (two d) -> p two d", two=2),
                    in_=wT[2 * j * _P : (2 * j + 2) * _P, :].rearrange(
                        "(two p) d -> p two d", two=2
                    ),
                )
                wtiles[2 * j] = (wt, 0)
                wtiles[2 * j + 1] = (wt, _D)

            def w_slice(k, o):
                wt, base = wtiles[k]
                return wt[:, base + o * _P : base + (o + 1) * _P]

            # x: per k, one DMA covering TWO token blocks (2KB rows) on the
            # scalar HWDGE queue so load issue runs parallel to the w queue.
            xtiles = {}  # (bpair, k) -> tile
            for b in range(NB):
                t0 = b * _TOK_BLOCK
                bp, half = divmod(b, 2)
                if half == 0:
                    for k in range(KC):
                        t = xp.tile(
                            [_P, 2 * _TOK_BLOCK], cd, tag=f"x{k}", name=f"x{k}_{bp}"
                        )
                        nc.scalar.dma_start(
                            out=t[:],
                            in_=xT[
                                k * _P : (k + 1) * _P, t0 : t0 + 2 * _TOK_BLOCK
                            ],
                        )
                        xtiles[(bp, k)] = t

                def x_slice(k):
                    return xtiles[(bp, k)][
                        :, half * _TOK_BLOCK : (half + 1) * _TOK_BLOCK
                    ]

                def epilogue(o, pt):
                    # scale and residual are folded into the host weights
                    # (W' = scale*W + I), so the epilogue is a plain PSUM
                    # drain: DVE copy to SBUF, gpsimd SWDGE store.
                    yt = yp.tile([_P, _TOK_BLOCK], f32, tag="y", name=f"y{o}_{b}")
                    nc.vector.tensor_copy(yt[:], pt[:])
                    nc.sync.dma_start(
                        out=yT[o * _P : (o + 1) * _P, t0 : t0 + _TOK_BLOCK],
                        in_=yt[:],
                    )

                if b == 0:
                    # k-outer for the first block: consumption order matches
                    # DMA arrival order (w_k + x_k per step), so the PE
                    # starts after ~0.75 MB instead of the full working set.
                    pts = [
                        ps.tile([_P, _TOK_BLOCK], f32, tag=f"ps{o}", name=f"ps{o}_0")
                        for o in range(OC)
                    ]
                    for k in range(KC):
                        for o in range(OC):
                            nc.tensor.matmul(
                                pts[o][:],
                                lhsT=w_slice(k, o),
                                rhs=x_slice(k),
                                start=(k == 0),
                                stop=(k == KC - 1),
                            )
                            if k == KC - 1:
                                epilogue(o, pts[o])
                else:
                    # o-outer / k-inner for steady state: each 128-row
                    # output chunk finishes every 8 matmuls, so its PSUM
                    # drain pipelines behind the PE instead of bunching up
                    # after the block's last matmul.
                    for o in range(OC):
                        pt = ps.tile(
                            [_P, _TOK_BLOCK], f32, tag=f"ps{o}", name=f"ps{o}_{b}"
                        )
                        for k in range(KC):
                            nc.tensor.matmul(
                                pt[:],
                                lhsT=w_slice(k, o),
                                rhs=x_slice(k),
                                start=(k == 0),
                                stop=(k == KC - 1),
                            )
                        epilogue(o, pt)
    return nc


def _np_compute_dtype():
    if _COMPUTE == "bf16":
        import ml_dtypes

        return ml_dtypes.bfloat16
    return np.float32


def kernel(x, w, scale):
    _apply_tile_drain_patch()
    from concourse.bass_utils import run_bass_kernel_spmd

    x = np.asarray(x, dtype=np.float32)
    w = np.asarray(w, dtype=np.float32)
    scale = np.asarray(scale, dtype=np.float32).reshape(1)
    cdt = _np_compute_dtype()
    # Fold the scalar scale and the residual identity into the weights:
    # y^T = (scale*W + I) @ x^T  ==  (scale * (x @ W^T) + x)^T  exactly.
    wTp = (w.reshape(_D, _D).T * scale[0] + np.eye(_D, dtype=np.float32)).astype(cdt)

    in_maps = []
    for i in range(_N_CORES):
        xs = x[i * _TOK_PER_CORE : (i + 1) * _TOK_PER_CORE]
        in_maps.append(
            {
                "xT": np.ascontiguousarray(xs.T).astype(cdt),
                "wT": wTp,
            }
        )

    if "nc" not in _cache:
        _cache["nc"] = _build_nc()
    res = run_bass_kernel_spmd(_cache["nc"], in_maps, core_ids=list(range(_N_CORES)))

    out = np.empty((_N_TOKENS, _D), dtype=np.float32)
    for i in range(_N_CORES):
        out[i * _TOK_PER_CORE : (i + 1) * _TOK_PER_CORE] = res.results[i]["yT"].T
    return out



# revision 2
# speedup vs baseline: 1.2274x; 1.2274x over previous
"""Trainium2 Bass kernel for nn_AstraloraLayer: y = (x @ W^T) * scale + x.

x: [16384, 1024] f32, w: [1048576] f32 (W = w.reshape(1024, 1024)),
scale: [1] f32.  Data-parallel over 8 NeuronCores: each core takes 2048
tokens; w and scale are replicated; no collectives needed.

Device layout: everything is computed transposed (y^T = W' @ x^T) so the
contraction dim d lands on SBUF partitions for both matmul operands with
zero on-device transposes.

Mixed-precision hybrid (rel err ~1.7e-2 vs the 2e-2 budget; fp8 alone
measures 2.5e-2 which is over):
  - k-chunks 0..3 (xT/W rows 0..511) run in bf16; the scalar `scale` AND
    the residual identity for outputs o<4 are folded into these weights.
  - k-chunks 4..7 run as fp8e4 (e4m3) DoubleRow matmuls: two k-chunks per
    pass at 0.5 cycles/row, halving PE time for this half of the GEMM.
    Operands are pre-scaled by 32 on the host so W entries (std 1/32)
    clear the e4m3 subnormal floor; the whole PSUM is therefore scaled by
    32 (bf16 weights carry the same factor) and the host divides the
    output by 32 (exact, power of two).
  - outputs o>=4 can't take the identity fold (their diagonal blocks land
    in the fp8 chunks where quantizing 32+32w would cost ~6% on the
    residual), so their PSUM drain is a DVE tensor_tensor add of
    r32 = 32*bf16(x) instead of a copy.  Outputs o<4 drain as plain
    copies on the Scalar (ACT) engine, keeping DVE and ACT each at ~11us
    of epilogue work, well under the PE stream.
  - y is stored as bf16 (halves store traffic; ~1e-3 rel err), upcast and
    unscaled on the host.

Block 0 runs k-outer across 8 PSUM banks so PE consumption matches DMA
arrival order (the first matmul waits on one 256 KB w chunk + one x
chunk, not the full working set); steady-state blocks run o-outer/
k-inner so each output chunk's PSUM drain pipelines behind the PE
instead of bunching at block end.  Six throwaway matmuls on zeroed
tiles pre-warm the PE's HAM clock gate during the DMA lead-in (input
sems only fire ~8.5us in, after the DMA write-receipt round trip).
w/r32 loads + y stores issue on the sync HWDGE queue, x loads on the
scalar HWDGE queue (DMA issue costs ~0.6us per 128-descriptor
instruction — two queues double the feed rate).
"""

import numpy as np

_N_TOKENS = 16384
_D = 1024
_N_CORES = 8
_TOK_PER_CORE = _N_TOKENS // _N_CORES  # 2048
_TOK_BLOCK = 512
_P = 128
_KB = 4  # bf16 k-chunks (k 0..3)
_NPAIR = 2  # fp8 DoubleRow pairs covering k 4..7
_S = 32.0  # power-of-two operand pre-scale for the e4m3 chunks

_cache = {}


def _apply_tile_drain_patch():
    """This walrus build rejects any instruction carrying more than one
    sync wait ("Too many sync wait commands", CoreV3 setupSyncWait), but
    Tile's wait-assignment pass freely emits multi-wait instructions.
    Two patches:

    1. Wrap TileClockWait so that after assign_waits() every instruction
       with >1 wait keeps only its last wait, with the others moved onto
       freshly inserted same-engine NoOps placed just before it.
    2. Re-emit the TileContext exit drain the same way (it waits on every
       live semaphore at once and is created after assign_waits ran).
    """
    if _cache.get("patched"):
        return
    import bass_rust
    import concourse.mybir as mybir
    from concourse import tile
    from concourse.vector_clock import ScopedClock

    _Orig = tile.TileClockWait
    _counter = [0]

    def _split_multi_waits(ordered):
        for insts in ordered.values():
            out = []
            for inst in insts:
                si = inst.sync_info
                if si is not None and len(si.on_wait) > 1:
                    waits = list(si.on_wait)
                    for w in waits[:-1]:
                        _counter[0] += 1
                        nop = mybir.InstNoOp(
                            name=f"I-wsplit-{_counter[0]}", ins=[], outs=[]
                        )
                        nop.engine = inst.engine
                        nop.bass_nofuse = True
                        nop.sync_info = bass_rust.SyncInfo(
                            on_wait=[w], on_update=[]
                        )
                        out.append(nop)
                    si.on_wait = waits[-1:]
                out.append(inst)
            insts[:] = out

    class _SplitWaitClock:
        def __init__(self, tc, ordered, **kw):
            object.__setattr__(self, "_inner", _Orig(tc, ordered, **kw))
            object.__setattr__(self, "_ordered", ordered)

        def assign_waits(self, bb):
            r = self._inner.assign_waits(bb)
            _split_multi_waits(self._ordered)
            return r

        def __getattr__(self, n):
            return getattr(object.__getattribute__(self, "_inner"), n)

    tile.TileClockWait = _SplitWaitClock

    def _drain_and_barrier(self, tick_clock, wait_clock):
        drain_inst = self.nc.sync.drain()
        wait_clock.add_sem_waits(
            drain_inst.ins, ScopedClock({None: tick_clock.global_clock})
        )
        si = drain_inst.ins.sync_info
        if si is not None and len(si.on_wait) > 1:
            waits = list(si.on_wait)
            si.on_wait = waits[:1]
            for w in waits[1:]:
                nop = self.nc.sync.nop(nofuse=True, hint="drain_wait_spill")
                nop.ins.sync_info = bass_rust.SyncInfo(on_wait=[w], on_update=[])

        self.nc.all_engine_barrier()
        assert self.sems is not None
        popped = self.nc._tile_sem_poison_stack.pop()
        assert popped is self._sem_poison
        self.nc.clear_and_free_semaphores(list(self.sems.allocated().values()))
        self.nc.all_engine_barrier()

    tile.TileContext._drain_and_barrier = _drain_and_barrier
    _cache["patched"] = True


def _build_nc():
    import concourse.bass as bass
    import concourse.mybir as mybir
    from concourse import tile

    f32 = mybir.dt.float32
    bf16 = mybir.dt.bfloat16
    f8 = mybir.dt.float8e4
    DR = mybir.MatmulPerfMode.DoubleRow
    OC = _D // _P  # 8 output-row chunks
    NB = _TOK_PER_CORE // _TOK_BLOCK  # 4 token blocks
    NKSTEP = _KB + _NPAIR  # 6 PE passes per (block, o)

    nc = bass.Bass()
    xb = nc.declare_dram_parameter("xb", [_KB * _P, _TOK_PER_CORE], bf16, isOutput=False)
    x8_0 = nc.declare_dram_parameter("x8_0", [_P, 2, _TOK_PER_CORE], f8, isOutput=False)
    x8_1 = nc.declare_dram_parameter("x8_1", [_P, 2, _TOK_PER_CORE], f8, isOutput=False)
    wb = nc.declare_dram_parameter("wb", [_KB * _P, _D], bf16, isOutput=False)
    w8_0 = nc.declare_dram_parameter("w8_0", [_P, 2, _D], f8, isOutput=False)
    w8_1 = nc.declare_dram_parameter("w8_1", [_P, 2, _D], f8, isOutput=False)
    r32 = nc.declare_dram_parameter("r32", [(OC - _KB) * _P, _TOK_PER_CORE], bf16, isOutput=False)
    yT = nc.declare_dram_parameter("yT", [_D, _TOK_PER_CORE], bf16, isOutput=True)
    x8d = [x8_0, x8_1]
    w8d = [w8_0, w8_1]

    with tile.TileContext(nc) as tc:
        with (
            tc.tile_pool(name="wp", bufs=1) as wp,
            tc.tile_pool(name="rp", bufs=1) as rp,
            tc.tile_pool(name="xp", bufs=2) as xp,
            tc.tile_pool(name="yp", bufs=8) as yp,
            tc.tile_pool(name="ps", bufs=1, space="PSUM") as ps,
        ):
            # PE pre-warm: the HAM clock gate holds the PE at 1.2 GHz until
            # it has been busy ~3.4us.  Input data only becomes sem-visible
            # ~8.5us in (DMA completion sems fire after the ~2us write
            # receipt, not at last byte), so six throwaway matmuls on zeroed
            # tiles keep the PE busy during the lead-in and it is at 2.4 GHz
            # when the real stream starts.
            warm_w = wp.tile([_P, _P], bf16, tag="warm_w")
            warm_x = wp.tile([_P, _TOK_BLOCK], bf16, tag="warm_x")
            nc.vector.memset(warm_w[:], 0.0)
            nc.vector.memset(warm_x[:], 0.0)
            warm_ps = ps.tile([_P, _TOK_BLOCK], f32, tag="ps7", name="warm_ps")
            for i in range(6):
                nc.tensor.matmul(
                    warm_ps[:], lhsT=warm_w[:], rhs=warm_x[:],
                    start=True, stop=True,
                )

            # bf16 weights (k 0..3): k0/k1 load as singles so the very first
            # matmul only waits on a 256 KB chunk; k2+k3 ride one batched DMA.
            wtiles = {}
            for k in (0, 1):
                wt = wp.tile([_P, _D], bf16, tag=f"ws{k}", name=f"ws{k}")
                nc.sync.dma_start(out=wt[:], in_=wb[k * _P : (k + 1) * _P, :])
                wtiles[k] = (wt, 0)
            wt = wp.tile([_P, 2 * _D], bf16, tag="w23", name="w23")
            nc.sync.dma_start(
                out=wt[:].rearrange("p (two d) -> p two d", two=2),
                in_=wb[2 * _P : 4 * _P, :].rearrange("(two p) d -> p two d", two=2),
            )
            wtiles[2] = (wt, 0)
            wtiles[3] = (wt, _D)

            # fp8 DoubleRow weights: [128, 2, 1024] per pair, host-packed.
            w8tiles = []
            for j in range(_NPAIR):
                w8t = wp.tile([_P, 2, _D], f8, tag=f"w8_{j}", name=f"w8_{j}")
                nc.sync.dma_start(out=w8t[:], in_=w8d[j][:, :, :])
                w8tiles.append(w8t)

            # Residual operand for o>=4: r32 = 32*bf16(x^T rows 512..1023),
            # one [128, 2048] tile per o-chunk, loaded once for all blocks.
            rtiles = []
            for i in range(OC - _KB):
                rt = rp.tile([_P, _TOK_PER_CORE], bf16, tag=f"r{i}", name=f"r{i}")
                nc.sync.dma_start(out=rt[:], in_=r32[i * _P : (i + 1) * _P, :])
                rtiles.append(rt)

            def w_slice(kstep, o):
                if kstep < _KB:
                    wt, base = wtiles[kstep]
                    return wt[:, base + o * _P : base + (o + 1) * _P]
                return w8tiles[kstep - _KB][:, :, o * _P : (o + 1) * _P]

            # x: per bf16 k (and per fp8 pair), one DMA covering TWO token
            # blocks on the scalar HWDGE queue so load issue runs parallel
            # to the w/r/store queue.
            xtiles = {}
            x8tiles = {}
            for b in range(NB):
                t0 = b * _TOK_BLOCK
                bp, half = divmod(b, 2)
                if half == 0:
                    tp0 = bp * 2 * _TOK_BLOCK
                    for k in range(_KB):
                        t = xp.tile(
                            [_P, 2 * _TOK_BLOCK], bf16, tag=f"x{k}", name=f"x{k}_{bp}"
                        )
                        nc.scalar.dma_start(
                            out=t[:],
                            in_=xb[k * _P : (k + 1) * _P, tp0 : tp0 + 2 * _TOK_BLOCK],
                        )
                        xtiles[(bp, k)] = t
                    for j in range(_NPAIR):
                        t = xp.tile(
                            [_P, 2, 2 * _TOK_BLOCK], f8, tag=f"x8_{j}",
                            name=f"x8_{j}_{bp}",
                        )
                        nc.scalar.dma_start(
                            out=t[:],
                            in_=x8d[j][:, :, tp0 : tp0 + 2 * _TOK_BLOCK],
                        )
                        x8tiles[(bp, j)] = t

                def x_slice(kstep):
                    lo = half * _TOK_BLOCK
                    hi = lo + _TOK_BLOCK
                    if kstep < _KB:
                        return xtiles[(bp, kstep)][:, lo:hi]
                    return x8tiles[(bp, kstep - _KB)][:, :, lo:hi]

                def mm(pt, kstep, o):
                    nc.tensor.matmul(
                        pt[:],
                        lhsT=w_slice(kstep, o),
                        rhs=x_slice(kstep),
                        start=(kstep == 0),
                        stop=(kstep == NKSTEP - 1),
                        perf_mode=(DR if kstep >= _KB else None),
                    )

                def epilogue(o, pt):
                    yt = yp.tile([_P, _TOK_BLOCK], bf16, tag="y", name=f"y{o}_{b}")
                    if o < _KB:
                        # residual identity folded into the bf16 weights;
                        # plain scaled-PSUM drain on the ACT engine.
                        nc.scalar.copy(yt[:], pt[:])
                    else:
                        nc.vector.tensor_tensor(
                            yt[:], pt[:],
                            rtiles[o - _KB][:, t0 : t0 + _TOK_BLOCK],
                            mybir.AluOpType.add,
                        )
                    nc.sync.dma_start(
                        out=yT[o * _P : (o + 1) * _P, t0 : t0 + _TOK_BLOCK],
                        in_=yt[:],
                    )

                if b == 0:
                    # k-outer for the first block: consumption order matches
                    # DMA arrival order (w_k + x_k per step), so the PE
                    # starts after ~0.5 MB instead of the full working set.
                    pts = [
                        ps.tile([_P, _TOK_BLOCK], f32, tag=f"ps{o}", name=f"ps{o}_0")
                        for o in range(OC)
                    ]
                    for kstep in range(NKSTEP):
                        for o in range(OC):
                            mm(pts[o], kstep, o)
                            if kstep == NKSTEP - 1:
                                epilogue(o, pts[o])
                else:
                    # o-outer / k-inner for steady state: each 128-row
                    # output chunk finishes every 6 PE passes, so its PSUM
                    # drain pipelines behind the PE instead of bunching up
                    # after the block's last matmul.
                    for o in range(OC):
                        pt = ps.tile(
                            [_P, _TOK_BLOCK], f32, tag=f"ps{o}", name=f"ps{o}_{b}"
                        )
                        for kstep in range(NKSTEP):
                            mm(pt, kstep, o)
                        epilogue(o, pt)
    return nc


def kernel(x, w, scale):
    _apply_tile_drain_patch()
    import ml_dtypes
    from concourse.bass_utils import run_bass_kernel_spmd

    bf16 = ml_dtypes.bfloat16
    e4m3 = ml_dtypes.float8_e4m3

    x = np.asarray(x, dtype=np.float32)
    w = np.asarray(w, dtype=np.float32)
    scale = np.asarray(scale, dtype=np.float32).reshape(1)

    KBROWS = _KB * _P  # 512

    # Weights, transposed and pre-scaled by 32 (exact power of two):
    #   PSUM = 32 * (scale * (x @ W^T) [+ x for o<4])^T
    WT32 = w.reshape(_D, _D).T * np.float32(_S * scale[0])
    wb_np = WT32[:KBROWS].copy()
    wb_np[:KBROWS, :KBROWS] += np.float32(_S) * np.eye(KBROWS, dtype=np.float32)
    wb_np = wb_np.astype(bf16)
    # fp8 pairs, packed [128, 2, 1024]: [p, i, m] = WT32[512 + (2j+i)*128 + p, m]
    w8_np = (
        WT32[KBROWS:].astype(e4m3)
        .reshape(_NPAIR, 2, _P, _D)
        .transpose(0, 2, 1, 3)
        .copy()
    )

    in_maps = []
    for i in range(_N_CORES):
        xT = np.ascontiguousarray(x[i * _TOK_PER_CORE : (i + 1) * _TOK_PER_CORE].T)
        x8 = (
            xT[KBROWS:].astype(e4m3)
            .reshape(_NPAIR, 2, _P, _TOK_PER_CORE)
            .transpose(0, 2, 1, 3)
            .copy()
        )
        in_maps.append(
            {
                "xb": xT[:KBROWS].astype(bf16),
                "x8_0": x8[0],
                "x8_1": x8[1],
                "wb": wb_np,
                "w8_0": w8_np[0],
                "w8_1": w8_np[1],
                "r32": (xT[KBROWS:] * np.float32(_S)).astype(bf16),
            }
        )

    if "nc" not in _cache:
        _cache["nc"] = _build_nc()
    res = run_bass_kernel_spmd(_cache["nc"], in_maps, core_ids=list(range(_N_CORES)))

    out = np.empty((_N_TOKENS, _D), dtype=np.float32)
    inv = np.float32(1.0 / _S)
    for i in range(_N_CORES):
        yt = res.results[i]["yT"].astype(np.float32) * inv
        out[i * _TOK_PER_CORE : (i + 1) * _TOK_PER_CORE] = yt.T
    return out
